# revision 4
# baseline (speedup 1.0000x reference)
"""Trainium2 Bass kernel for the sparse-attention (entmax-gated causal attention) module.

Sharding: 8 cores = 4 batches x 2 head-groups (8 heads each).
Each core computes, for its batch b and head group g:
  - the full (T,T) gate matrix (cumprobs / attn_mask) in transposed [s,t] layout
    (replicated across the 2 head-group cores of the batch; host reads it from the even core)
  - attention output for its 8 heads and the partial y projection against its
    512-column slice of W_proj (host sums the two partials per batch).

The entmax_bisect(alpha=1.000001) over [s, 0] equals sigmoid(s) up to the
bisection's float32 rounding noise; we compute sigmoid directly.

Matmuls run in fp16 (1 cycle/row on PE vs 4 for fp32; 10-bit mantissa keeps
relative error ~5e-4, well under the sigmoid-vs-bisection noise floor).
The cumprod scan, sigmoid, ln and all (T,T) outputs stay fp32.
"""

import math
import numpy as np
from contextlib import ExitStack

import concourse.bass as bass
import concourse.mybir as mybir
import concourse.tile as tile
from concourse import bacc
from concourse.bass import ts, ds
from concourse.bass_utils import run_bass_kernel_spmd

B, T, C = 4, 1024, 1024
H_PER_CORE = 8
HD = 64
DI = 64
P = 128
N_CORES = 8
FP = mybir.dt.float32
MM = mybir.dt.float16
MM_NP = np.float16

AF = mybir.ActivationFunctionType
ALU = mybir.AluOpType
NEG_INF = float("-inf")


def build_program():
    nc = bacc.Bacc("TRN2", target_bir_lowering=False, debug=False)

    xT = nc.dram_tensor("xT", [C, T], MM, kind="ExternalInput").ap()
    wqkT = nc.dram_tensor("wqkT", [C, 1024], MM, kind="ExternalInput").ap()
    wvT = nc.dram_tensor("wvT", [C, 512], MM, kind="ExternalInput").ap()
    wpT = nc.dram_tensor("wpT", [512, C], MM, kind="ExternalInput").ap()
    wiT = nc.dram_tensor("wiT", [C, 128], MM, kind="ExternalInput").ap()
    biascol = nc.dram_tensor("biascol", [P, 1], FP, kind="ExternalInput").ap()

    ypart = nc.dram_tensor("ypart", [T, C], FP, kind="ExternalOutput").ap()
    cumprobsT = nc.dram_tensor("cumprobsT", [T, T], FP, kind="ExternalOutput").ap()
    maskT = nc.dram_tensor("maskT", [T, T], FP, kind="ExternalOutput").ap()

    with tile.TileContext(nc) as tc:
        _body(tc, xT, wqkT, wvT, wpT, wiT, biascol, ypart, cumprobsT, maskT)
    nc.compile()
    return nc


def _body(tc, xT, wqkT, wvT, wpT, wiT, biascol, ypart, cumprobsT, maskT):
    nc = tc.nc
    mm = nc.tensor.matmul
    with ExitStack() as top:
        const = top.enter_context(tc.tile_pool(name="const", bufs=1))
        biascol_sb = const.tile([P, 1], FP)
        nc.sync.dma_start(out=biascol_sb[:], in_=biascol[:])
        ones_sb = const.tile([1, 64], MM)
        nc.any.memset(ones_sb[:], 1.0)

        x_pool = top.enter_context(tc.tile_pool(name="x", bufs=1))
        xT_sb = x_pool.tile([P, 8, T], MM)
        nc.sync.dma_start(out=xT_sb[:], in_=xT.rearrange("(c p) t -> p c t", p=P))

        cg_pool = top.enter_context(tc.tile_pool(name="cg", bufs=1))
        cgT_sb = cg_pool.tile([P, 8, T], MM)  # causal-gated cumprobs, [s, t]

        # ---------------- gate phase ----------------
        with ExitStack() as gph:
            wi_pool = gph.enter_context(tc.tile_pool(name="wi", bufs=1))
            wiT_sb = wi_pool.tile([P, 8, 128], MM)
            nc.sync.dma_start(out=wiT_sb[:], in_=wiT.rearrange("(c p) f -> p c f", p=P))

            int_pool = gph.enter_context(tc.tile_pool(name="intT", bufs=1))
            qiT_sb = int_pool.tile([64, T], MM, tag="qiT")
            kiT_sb = int_pool.tile([64, T], MM, tag="kiT")

            psum_g = gph.enter_context(tc.tile_pool(name="psum_g", bufs=2, space="PSUM"))
            for fi, dst in ((0, qiT_sb), (1, kiT_sb)):
                for th in range(2):
                    pg = psum_g.tile([64, 512], FP, tag="pg_int")
                    for cc in range(8):
                        mm(pg[:], wiT_sb[:, cc, ts(fi, 64)],
                           xT_sb[:, cc, ts(th, 512)],
                           start=(cc == 0), stop=(cc == 7))
                    nc.scalar.copy(dst[:, ts(th, 512)], pg[:])

            gate_cp = gph.enter_context(tc.tile_pool(name="gate_cp", bufs=8))
            gate_wk = gph.enter_context(tc.tile_pool(name="gate_wk", bufs=3))
            cp_tiles = []
            # loop A: sigmoid gate + cumprod scan (ACT table: sigmoid)
            for sc in range(8):
                s0 = sc * 128
                pT = gate_wk.tile([P, T], FP, tag="pT")
                for th in range(2):
                    pg = psum_g.tile([P, 512], FP, tag="pg_G")
                    mm(pg[:], kiT_sb[:, ts(sc, 128)], qiT_sb[:, ts(th, 512)])
                    nc.scalar.activation(pT[:, ts(th, 512)], pg[:], AF.Sigmoid,
                                         bias=biascol_sb[:], scale=1.0 / math.sqrt(DI))
                # p = 1 where t <= s
                if s0 > 0:
                    nc.vector.memset(pT[:, 0:s0], 1.0)
                nc.gpsimd.affine_select(pT[:, ds(s0, 128)], pT[:, ds(s0, 128)],
                                        [[1, 128]], ALU.is_ge, 1.0,
                                        base=-1, channel_multiplier=-1)
                cp = gate_cp.tile([P, T], FP)
                cp_tiles.append(cp)
                nc.vector.tensor_tensor_scan(cp[:], pT[:], pT[:], 1.0,
                                             op0=ALU.mult, op1=ALU.bypass)
                nc.sync.dma_start(out=cumprobsT[ds(s0, 128), :], in_=cp[:])
                # gated fp16 copy for attention: cg = cp where t >= s else 0
                if s0 > 0:
                    nc.vector.memset(cgT_sb[:, sc, 0:s0], 0.0)
                nc.vector.tensor_copy(cgT_sb[:, sc, ds(s0, 128)], cp[:, ds(s0, 128)])
                nc.gpsimd.affine_select(cgT_sb[:, sc, ds(s0, 128)],
                                        cgT_sb[:, sc, ds(s0, 128)],
                                        [[1, 128]], ALU.is_ge, 0.0,
                                        base=0, channel_multiplier=-1)
                if s0 + 128 < T:
                    nc.vector.tensor_copy(cgT_sb[:, sc, ds(s0 + 128, T - s0 - 128)],
                                          cp[:, ds(s0 + 128, T - s0 - 128)])
            # loop B: attn_mask = ln(cumprobs), -inf above diagonal (ACT table: ln/exp)
            for sc in range(8):
                s0 = sc * 128
                cp = cp_tiles[sc]
                mk = gate_wk.tile([P, T], FP, tag="mk")
                nc.scalar.activation(mk[:], cp[:], AF.Ln)
                if s0 > 0:
                    nc.vector.memset(mk[:, 0:s0], NEG_INF)
                nc.gpsimd.affine_select(mk[:, ds(s0, 128)], mk[:, ds(s0, 128)],
                                        [[1, 128]], ALU.is_ge, NEG_INF,
                                        base=0, channel_multiplier=-1)
                nc.sync.dma_start(out=maskT[ds(s0, 128), :], in_=mk[:])

        # ---------------- qkv projections ----------------
        qk_pool = top.enter_context(tc.tile_pool(name="qkT", bufs=1))
        qkT_sb = qk_pool.tile([P, 8, T], MM)  # feature rows: 0-511 q, 512-1023 k
        v_pool = top.enter_context(tc.tile_pool(name="v1", bufs=1))
        v1_sb = v_pool.tile([P, 8, H_PER_CORE, 65], MM)  # [t%128, t_chunk, head, d+ones]
        nc.any.memset(v1_sb[:, :, :, 64:65], 1.0)

        with ExitStack() as qph:
            wqk_pool = qph.enter_context(tc.tile_pool(name="wqk", bufs=1))
            wqkT_sb = wqk_pool.tile([P, 8, 1024], MM)
            nc.sync.dma_start(out=wqkT_sb[:], in_=wqkT.rearrange("(c p) f -> p c f", p=P))
            wv_pool = qph.enter_context(tc.tile_pool(name="wv", bufs=1))
            wvT_sb = wv_pool.tile([P, 8, 512], MM)
            nc.sync.dma_start(out=wvT_sb[:], in_=wvT.rearrange("(c p) f -> p c f", p=P))

            psum_mm = qph.enter_context(tc.tile_pool(name="psum_mm", bufs=4, space="PSUM"))
            for rc in range(8):
                for th in range(2):
                    pg = psum_mm.tile([P, 512], FP)
                    for cc in range(8):
                        mm(pg[:], wqkT_sb[:, cc, ts(rc, 128)],
                           xT_sb[:, cc, ts(th, 512)],
                           start=(cc == 0), stop=(cc == 7))
                    nc.scalar.copy(qkT_sb[:, rc, ts(th, 512)], pg[:])
            for tch in range(8):
                pg = psum_mm.tile([P, 512], FP)
                for cc in range(8):
                    mm(pg[:], xT_sb[:, cc, ts(tch, 128)], wvT_sb[:, cc, :],
                       start=(cc == 0), stop=(cc == 7))
                nc.vector.tensor_copy(v1_sb[:, tch, :, 0:64], pg[:])

        # ---------------- attention ----------------
        oT_pool = top.enter_context(tc.tile_pool(name="oT", bufs=1))
        oT_sb = oT_pool.tile([P, 4, T], MM)  # y_local^T: c_local = chunk*128 + p

        wp_pool = top.enter_context(tc.tile_pool(name="wp", bufs=1))
        wpT_sb = wp_pool.tile([P, 4, 1024], MM)
        nc.sync.dma_start(out=wpT_sb[:], in_=wpT.rearrange("(c p) j -> p c j", p=P))

        with ExitStack() as aph:
            att_e = aph.enter_context(tc.tile_pool(name="att_e", bufs=4))
            ps_pool = aph.enter_context(tc.tile_pool(name="ps_s", bufs=2, space="PSUM"))
            po_pool = aph.enter_context(tc.tile_pool(name="ps_o", bufs=2, space="PSUM"))
            pb_pool = aph.enter_context(tc.tile_pool(name="ps_b", bufs=2, space="PSUM"))
            rec_pool = aph.enter_context(tc.tile_pool(name="rec", bufs=2))

            for h in range(H_PER_CORE):
                prow = (h % 2) * 64
                qT = qkT_sb[prow:prow + 64, h // 2, :]
                kT = qkT_sb[prow:prow + 64, 4 + h // 2, :]
                for th in range(2):
                    nsc = 4 * (th + 1)
                    po = po_pool.tile([65, 512], FP)
                    for sc in range(nsc):
                        ps = ps_pool.tile([P, 512], FP)
                        mm(ps[:], kT[:, ts(sc, 128)], qT[:, ts(th, 512)])
                        e = att_e.tile([P, 512], MM)
                        nc.scalar.activation(e[:], ps[:], AF.Exp, scale=1.0 / math.sqrt(HD))
                        nc.vector.tensor_mul(e[:], e[:], cgT_sb[:, sc, ts(th, 512)])
                        mm(po[:], v1_sb[:, sc, h, :], e[:],
                           start=(sc == 0), stop=(sc == nsc - 1))
                    r = rec_pool.tile([1, 512], FP, tag="r")
                    nc.vector.reciprocal(r[:], po[64:65, :])
                    r16 = rec_pool.tile([1, 512], MM, tag="r16")
                    nc.vector.tensor_copy(r16[:], r[:])
                    pb = pb_pool.tile([64, 512], FP)
                    mm(pb[:], ones_sb[:], r16[:])
                    rb = rec_pool.tile([64, 512], FP, tag="rb")
                    nc.scalar.copy(rb[:], pb[:])
                    nc.vector.tensor_mul(oT_sb[prow:prow + 64, h // 2, ts(th, 512)],
                                         po[0:64, :], rb[:])

        # ---------------- output projection (partial over local c) ----------------
        with ExitStack() as pph:
            psum_y = pph.enter_context(tc.tile_pool(name="psum_y", bufs=4, space="PSUM"))
            y_pool = pph.enter_context(tc.tile_pool(name="y", bufs=4))
            for tch in range(8):
                for jh in range(2):
                    pg = psum_y.tile([P, 512], FP)
                    for cc in range(4):
                        mm(pg[:], oT_sb[:, cc, ts(tch, 128)],
                           wpT_sb[:, cc, ts(jh, 512)],
                           start=(cc == 0), stop=(cc == 3))
                    y = y_pool.tile([P, 512], FP)
                    nc.scalar.copy(y[:], pg[:])
                    nc.sync.dma_start(out=ypart[ts(tch, 128), ts(jh, 512)], in_=y[:])


def host_prep(x, W_attn, W_proj, Wq_int, Wk_int, int_bias):
    x = np.asarray(x, dtype=np.float32)
    W_attn = np.asarray(W_attn, dtype=np.float32)
    W_proj = np.asarray(W_proj, dtype=np.float32)
    wiT = np.ascontiguousarray(
        np.concatenate([np.asarray(Wq_int, np.float32).T,
                        np.asarray(Wk_int, np.float32).T], axis=1)).astype(MM_NP)
    biascol = np.full((P, 1), np.float32(np.asarray(int_bias).reshape(-1)[0]),
                      dtype=np.float32)
    in_maps = []
    for core in range(N_CORES):
        b, g = core // 2, core % 2
        qrows = W_attn[g * 512:(g + 1) * 512]
        krows = W_attn[1024 + g * 512:1024 + (g + 1) * 512]
        vrows = W_attn[2048 + g * 512:2048 + (g + 1) * 512]
        in_maps.append({
            "xT": np.ascontiguousarray(x[b].T).astype(MM_NP),
            "wqkT": np.ascontiguousarray(np.concatenate([qrows, krows], axis=0).T).astype(MM_NP),
            "wvT": np.ascontiguousarray(vrows.T).astype(MM_NP),
            "wpT": np.ascontiguousarray(W_proj[:, g * 512:(g + 1) * 512].T).astype(MM_NP),
            "wiT": wiT,
            "biascol": biascol,
        })
    return in_maps


def host_gather(results):
    y = np.empty((B, T, C), np.float32)
    cumprobs = np.empty((B, 1, T, T), np.float32)
    attn_mask = np.empty((B, 1, T, T), np.float32)
    for b in range(B):
        y[b] = results[2 * b]["ypart"] + results[2 * b + 1]["ypart"]
        cumprobs[b, 0] = results[2 * b]["cumprobsT"].T
        attn_mask[b, 0] = results[2 * b]["maskT"].T
    return y, cumprobs, attn_mask


_NC_CACHE = {}


def kernel(x, W_attn, W_proj, Wq_int, Wk_int, int_bias):
    if "nc" not in _NC_CACHE:
        _NC_CACHE["nc"] = build_program()
    nc = _NC_CACHE["nc"]
    in_maps = host_prep(x, W_attn, W_proj, Wq_int, Wk_int, int_bias)
    res = run_bass_kernel_spmd(nc, in_maps, list(range(N_CORES)))
    return host_gather(res.results)
